# revision 15
# baseline (speedup 1.0000x reference)
"""CrossGender InfoNCE loss on 8 trn2 NeuronCores.

Math: for direction f->m (anchor rows f_i, positives m_j, all L2-normalized):
    P_i = sum_j exp((f_i . m_j - 1)/T)        (numerator sum, shifted)
    N_i = sum_{j!=i} exp((f_i . f_j - 1)/T)   (masked self-sim sum, shifted)
    per-row term = log(P_i + N_i) - log(P_i)  (shift cancels in the ratio)
    loss_f2m = mean_i term_i ; loss = 1.5*loss_f2m + 0.5*loss_m2f

Sharding: core c owns anchor rows [1024c, 1024c+1024) of both proj_f and
proj_m.  Inputs are shipped transposed ([D=128, N=8192], bf16).  ft/mt are
column-rotated by 1024c so each core's own block sits at column 0 — the
self-similarity diagonal then lands at a uniform (core-independent)
position, so one SPMD program masks it exactly (subtract 30000*I on PSUM
before exp).  The cross-gender sim matrix F.M^T is computed ONCE per core
(strip A, unrotated M columns): row-sums give P for the core's F-anchors;
bf16 exp tiles are column-accumulated on the DVE, partition-reduced with a
ones-matmul, and ReduceScattered (add) across cores — in global column
order rank c's RS shard is exactly Q for its own M-anchor rows.  That kills
one of the four exp strips (the ACT engine is the bottleneck at
~1 elem/lane/cycle).  Each core emits two partial sums; the host combines
8x2 scalars.
"""

import numpy as np

N_ROWS = 8192
D = 128
N_CORES = 8
ROWS_PER_CORE = N_ROWS // N_CORES  # 1024
SUB = 128                          # anchor sub-block (psum partition dim)
NB = ROWS_PER_CORE // SUB          # 8 sub-blocks per core
CHUNK = 2048                       # psum columns per ACT pass (4 banks)
NCH = N_ROWS // CHUNK              # 4 chunks
MM_N = 512                         # matmul moving free dim (1 psum bank)
TEMPERATURE = 0.07
ASYM_F = 1.5
ASYM_M = 0.5
MASK_BIG = 30000.0


def _split_waits(nc, mybir, maxw=1):
    """Workaround: this walrus build rejects >1 sync-wait on one instruction
    ("Too many sync wait commands").  Hoist extras onto preceding NoOps on
    the same engine (engines execute sequentially, so semantics keep)."""
    fn = nc.m.functions[0]
    n_new = 0
    for blk in fn.blocks:
        out = []
        changed = False
        for inst in blk.instructions:
            si = inst.sync_info
            if si is not None and si.on_wait and len(si.on_wait) > maxw:
                waits = list(si.on_wait)
                for w in waits[:-maxw]:
                    nop = mybir.InstNoOp(
                        name=f"{inst.name}-w{n_new}", ins=[], outs=[]
                    )
                    n_new += 1
                    nop.engine = inst.engine
                    nop.sync_info = mybir.SyncInfo(on_wait=[w], on_update=[])
                    out.append(nop)
                inst.sync_info = mybir.SyncInfo(
                    on_wait=waits[-maxw:], on_update=list(si.on_update)
                )
                changed = True
            out.append(inst)
        if changed:
            blk.instructions = out
    return n_new


def build_program(repeats=1):
    """Build the SPMD Bass module (same program for all 8 cores)."""
    from contextlib import ExitStack

    import concourse.bass as bass
    import concourse.tile as tile
    from concourse import mybir

    f32 = mybir.dt.float32
    bf16 = mybir.dt.bfloat16
    AF = mybir.ActivationFunctionType
    ALU = mybir.AluOpType

    nc = bass.Bass(
        "TRN2", target_bir_lowering=False, debug=False, num_devices=N_CORES
    )
    ft_d = nc.dram_tensor("ft", [D, N_ROWS], bf16, kind="ExternalInput")
    mt_d = nc.dram_tensor("mt", [D, N_ROWS], bf16, kind="ExternalInput")
    mg_d = nc.dram_tensor("mg", [D, N_ROWS], bf16, kind="ExternalInput")
    eye_d = nc.dram_tensor("eye", [SUB, SUB], f32, kind="ExternalInput")
    out_d = nc.dram_tensor("partials", [1, 2], f32, kind="ExternalOutput")

    scale = 1.0 / TEMPERATURE
    bias = -1.0 / TEMPERATURE

    with tile.TileContext(nc) as tc:
        with ExitStack() as ctx:
            data = ctx.enter_context(tc.tile_pool(name="data", bufs=1))
            psum = ctx.enter_context(
                tc.tile_pool(name="psum", bufs=2, space="PSUM")
            )
            scratch = ctx.enter_context(tc.tile_pool(name="scratch", bufs=3))
            csump = ctx.enter_context(tc.tile_pool(name="csump", bufs=1))
            statp = ctx.enter_context(tc.tile_pool(name="statp", bufs=1))
            dramp = ctx.enter_context(
                tc.tile_pool(name="dramp", bufs=2, space="DRAM")
            )

            def body(rep):
                ft_s = data.tile([D, N_ROWS], bf16, name="ft_s")
                mt_s = data.tile([D, N_ROWS], bf16, name="mt_s")
                mg_s = data.tile([D, N_ROWS], bf16, name="mg_s")
                eye_s = data.tile([SUB, SUB], f32, name="eye_s")
                bias_s = data.tile([D, 1], f32, name="bias_s")
                onesf = data.tile([D, 1], f32, name="onesf")
                nc.vector.memset(bias_s, bias)
                nc.vector.memset(onesf, 1.0)
                # chunk-0 operands first so the first matmuls start early
                nc.sync.dma_start(out=ft_s[:, 0:CHUNK], in_=ft_d[:, 0:CHUNK])
                nc.sync.dma_start(out=mg_s[:, 0:CHUNK], in_=mg_d[:, 0:CHUNK])
                for k in range(1, NCH):
                    sl = slice(k * CHUNK, (k + 1) * CHUNK)
                    nc.sync.dma_start(out=mg_s[:, sl], in_=mg_d[:, sl])
                    nc.sync.dma_start(out=ft_s[:, sl], in_=ft_d[:, sl])
                for k in range(NCH):
                    sl = slice(k * CHUNK, (k + 1) * CHUNK)
                    nc.sync.dma_start(out=mt_s[:, sl], in_=mt_d[:, sl])
                nc.sync.dma_start(out=eye_s, in_=eye_d[:, :])

                # stats[p, strip, b, k] = rowsum of exp over that chunk
                stats = statp.tile([D, 3, NB, NCH], f32, name="stats")
                csvec = statp.tile([1, N_ROWS], f32, name="csvec")

                def sim_chunk(istrip, anchor, target, b, k, masked):
                    """matmul 128 anchors x CHUNK targets -> exp -> rowsums;
                    returns the bf16 exp tile."""
                    lhsT = anchor[:, b * SUB : (b + 1) * SUB]
                    ps = psum.tile([D, CHUNK], f32, name="ps", tag="ps")
                    for s in range(CHUNK // MM_N):
                        nc.tensor.matmul(
                            ps[:, s * MM_N : (s + 1) * MM_N],
                            lhsT,
                            target[
                                :,
                                k * CHUNK + s * MM_N : k * CHUNK + (s + 1) * MM_N,
                            ],
                            start=True,
                            stop=True,
                        )
                    if masked and k == 0:
                        w = b * SUB
                        nc.vector.scalar_tensor_tensor(
                            out=ps[:, w : w + SUB],
                            in0=ps[:, w : w + SUB],
                            scalar=1.0,
                            in1=eye_s,
                            op0=ALU.bypass,
                            op1=ALU.subtract,
                        )
                    sc = scratch.tile([D, CHUNK], bf16, name="sc")
                    nc.scalar.activation(
                        out=sc,
                        in_=ps,
                        func=AF.Exp,
                        bias=bias_s,
                        scale=scale,
                        accum_out=stats[:, istrip, b, k : k + 1],
                    )
                    return sc

                # --- strip A: F anchors vs global-order M columns -------
                # row-sums -> P ; column accumulation -> Q partials
                csums = []
                for k in range(NCH):
                    csum = csump.tile([D, CHUNK], f32, name="csum", tag=f"csum{k}")
                    csums.append(csum)
                    for b in range(NB):
                        sc = sim_chunk(0, ft_s, mg_s, b, k, masked=False)
                        if b == 0:
                            nc.vector.tensor_copy(csum, sc)
                        else:
                            nc.vector.scalar_tensor_tensor(
                                out=csum, in0=csum, scalar=1.0, in1=sc,
                                op0=ALU.bypass, op1=ALU.add,
                            )
                # --- strips B/C: masked self-similarity ------------------
                # The colsum partition-reduce (16 fp32 ones-matmuls) is
                # spliced in after strip B's first sub-block: the in-order
                # PE then hides them in ACT-limited idle instead of gating
                # strip B's first psum fills.
                def colsum_reduce_and_rs():
                    cc_in = dramp.tile([1, N_ROWS], f32, name="cc_in")
                    cc_out = dramp.tile([1, ROWS_PER_CORE], f32, name="cc_out")
                    for k in range(NCH):
                        pcs = psum.tile([D, CHUNK], f32, name="pcs", tag="ps")
                        for s in range(CHUNK // MM_N):
                            nc.tensor.matmul(
                                pcs[0:1, s * MM_N : (s + 1) * MM_N],
                                onesf,
                                csums[k][:, s * MM_N : (s + 1) * MM_N],
                                start=True,
                                stop=True,
                            )
                        nc.vector.tensor_copy(
                            csvec[:, k * CHUNK : (k + 1) * CHUNK], pcs[0:1, :]
                        )
                    nc.gpsimd.dma_start(out=cc_in, in_=csvec)
                    nc.gpsimd.collective_compute(
                        "ReduceScatter",
                        mybir.AluOpType.add,
                        replica_groups=[list(range(N_CORES))],
                        ins=[cc_in[:, :]],
                        outs=[cc_out[:, :]],
                    )
                    # core's Q, laid out [part p, sub-block b] = rs[128b+p]
                    qsb = statp.tile([D, NB], f32, name="qsb")
                    nc.gpsimd.dma_start(
                        out=qsb,
                        in_=cc_out.rearrange("o (b p) -> p (o b)", p=SUB),
                    )
                    return qsb

                qsb = None
                for istrip, (anchor, target) in enumerate(
                    [(ft_s, ft_s), (mt_s, mt_s)], start=1
                ):
                    for b in range(NB):
                        for k in range(NCH):
                            sim_chunk(istrip, anchor, target, b, k, masked=True)
                        if istrip == 1 and b == 0:
                            qsb = colsum_reduce_and_rs()

                # --- finale: per-row sums -> per-core partial scalars ----
                sums = statp.tile([D, 3, NB], f32, name="sums")
                for istrip in range(3):
                    nc.vector.tensor_reduce(
                        out=sums[:, istrip, :],
                        in_=stats[:, istrip, :, :],
                        axis=mybir.AxisListType.X,
                        op=ALU.add,
                    )
                den = statp.tile([D, 2, NB], f32, name="den")
                nc.vector.scalar_tensor_tensor(
                    out=den[:, 0, :], in0=sums[:, 0, :], scalar=1.0,
                    in1=sums[:, 1, :], op0=ALU.bypass, op1=ALU.add,
                )
                nc.vector.scalar_tensor_tensor(
                    out=den[:, 1, :], in0=qsb, scalar=1.0,
                    in1=sums[:, 2, :], op0=ALU.bypass, op1=ALU.add,
                )
                # term sums per partition: sum_b [ln(den) - ln(num)]
                lnacc = statp.tile([D, 4], f32, name="lnacc")
                lnscr = statp.tile([D, NB], f32, name="lnscr")
                for i, src in enumerate(
                    [den[:, 0, :], sums[:, 0, :], den[:, 1, :], qsb]
                ):
                    nc.scalar.activation(
                        out=lnscr,
                        in_=src,
                        func=AF.Ln,
                        accum_out=lnacc[:, i : i + 1],
                    )
                term = statp.tile([D, 2], f32, name="term")
                nc.vector.scalar_tensor_tensor(
                    out=term[:, 0:1], in0=lnacc[:, 0:1], scalar=1.0,
                    in1=lnacc[:, 1:2], op0=ALU.bypass, op1=ALU.subtract,
                )
                nc.vector.scalar_tensor_tensor(
                    out=term[:, 1:2], in0=lnacc[:, 2:3], scalar=1.0,
                    in1=lnacc[:, 3:4], op0=ALU.bypass, op1=ALU.subtract,
                )
                # reduce across the 128 partitions with a ones-matmul
                pfin = psum.tile([D, CHUNK], f32, name="pfin", tag="ps")
                nc.tensor.matmul(
                    pfin[0:1, 0:2], onesf, term, start=True, stop=True
                )
                res = statp.tile([1, 2], f32, name="res")
                nc.vector.tensor_copy(res, pfin[0:1, 0:2])
                nc.gpsimd.dma_start(out=out_d[:, :], in_=res)

            for rep in range(repeats):
                body(rep)

    _split_waits(nc, mybir)
    return nc


def make_in_maps(proj_f, proj_m):
    import ml_dtypes

    ftT = np.ascontiguousarray(proj_f.astype(np.float32).T)  # [D, N]
    mtT = np.ascontiguousarray(proj_m.astype(np.float32).T)
    mg = mtT.astype(ml_dtypes.bfloat16)
    eye = MASK_BIG * np.eye(SUB, dtype=np.float32)
    in_maps = []
    for c in range(N_CORES):
        shift = c * ROWS_PER_CORE
        ftc = np.roll(ftT, -shift, axis=1).astype(ml_dtypes.bfloat16)
        mtc = np.roll(mtT, -shift, axis=1).astype(ml_dtypes.bfloat16)
        in_maps.append({"ft": ftc, "mt": mtc, "mg": mg, "eye": eye})
    return in_maps


def combine_partials(results):
    sum_f = 0.0
    sum_m = 0.0
    for r in results:
        p = np.asarray(r["partials"], dtype=np.float64)
        sum_f += p[0, 0]
        sum_m += p[0, 1]
    loss = ASYM_F * (sum_f / N_ROWS) + ASYM_M * (sum_m / N_ROWS)
    return np.float32(loss)


def kernel(proj_f, proj_m):
    from concourse.bass_utils import run_bass_kernel_spmd

    nc = build_program(repeats=1)
    in_maps = make_in_maps(proj_f, proj_m)
    res = run_bass_kernel_spmd(
        nc, in_maps, core_ids=list(range(N_CORES)), trace=False
    )
    return combine_partials(res.results)


# revision 29
# speedup vs baseline: 1.0081x; 1.0081x over previous
"""CrossGender InfoNCE loss on 8 trn2 NeuronCores.

Math: for direction f->m (anchor rows f_i, positives m_j, all L2-normalized):
    P_i = sum_j exp((f_i . m_j - 1)/T)        (numerator sum, shifted)
    N_i = sum_{j!=i} exp((f_i . f_j - 1)/T)   (masked self-sim sum, shifted)
    per-row term = log(P_i + N_i) - log(P_i)  (shift cancels in the ratio)
    loss_f2m = mean_i term_i ; loss = 1.5*loss_f2m + 0.5*loss_m2f

Sharding: core c owns anchor rows [1024c, 1024c+1024) of both proj_f and
proj_m.  Inputs are shipped transposed ([D=128, N=8192], bf16).  ft/mt are
column-rotated by 1024c so each core's own block sits at column 0; the
self-similarity diagonal then lands at a uniform (core-independent)
position, so one SPMD program masks it exactly (subtract 30000*I on PSUM
before exp).  The ACT (exp) engine is the bottleneck (1 elem/lane/cycle +
~350cyc/instruction), so exp work is shared three ways:

* strip A (F.M^T, unrotated M columns) is computed once: row-sums give P;
  column-sums are accumulated on the PE (ones-matmuls over the bf16 exp
  tiles into dedicated PSUM banks) and ReduceScattered (add) across cores —
  in global column order rank c's RS shard is exactly Q for its own
  M-anchor rows.
* strips B (F.F^T) and C (M.M^T) are symmetric: each core computes only
  rotated column blocks d=0..4 (5/8 of the row); blocks d=1..3 also feed
  column-sums that are AllGathered, and each core fetches the 3 segments
  addressed to it (partition-id register offsets) to complete its row-sums.
  Distance-4 blocks are computed by both endpoint cores (rowsum only), so
  nothing is exchanged for them.

All compute runs in 1024-column units: sim psum ping-pong = 2x2 banks,
colsum accumulators = 2x2 banks (8 total).  Colsum matmuls are emitted one
sub-block behind the sim matmuls so the in-order PE never makes the ACT
wait.  Exchange collectives fly under later strips' exp work.  Each core
emits two partial sums; the host combines 8x2 scalars.
"""

import numpy as np

N_ROWS = 8192
D = 128
N_CORES = 8
ROWS_PER_CORE = N_ROWS // N_CORES  # 1024
SUB = 128                          # anchor sub-block (psum partition dim)
NB = ROWS_PER_CORE // SUB          # 8 sub-blocks per core
UNIT = 1024                        # columns per ACT pass (2 psum banks)
MM_N = 512                         # matmul moving free dim
NAU = N_ROWS // UNIT               # 8 units (strip A)
NBCU = 5                           # strip B/C units (d=0..4)
XCOLS = 3 * ROWS_PER_CORE          # exchanged colsum width (d=1..3)
TEMPERATURE = 0.07
ASYM_F = 1.5
ASYM_M = 0.5
MASK_BIG = 30000.0
DVE_ROWSUM = False  # rowsums on the DVE instead of ACT accum_out (A/B knob)


def _split_waits(nc, mybir, maxw=1):
    """Workaround: this walrus build rejects >1 sync-wait on one instruction
    ("Too many sync wait commands").  Hoist extras onto preceding NoOps on
    the same engine (engines execute sequentially, so semantics keep)."""
    fn = nc.m.functions[0]
    n_new = 0
    for blk in fn.blocks:
        out = []
        changed = False
        for inst in blk.instructions:
            si = inst.sync_info
            if si is not None and si.on_wait and len(si.on_wait) > maxw:
                waits = list(si.on_wait)
                for w in waits[:-maxw]:
                    nop = mybir.InstNoOp(
                        name=f"{inst.name}-w{n_new}", ins=[], outs=[]
                    )
                    n_new += 1
                    nop.engine = inst.engine
                    nop.sync_info = mybir.SyncInfo(on_wait=[w], on_update=[])
                    out.append(nop)
                inst.sync_info = mybir.SyncInfo(
                    on_wait=waits[-maxw:], on_update=list(si.on_update)
                )
                changed = True
            out.append(inst)
        if changed:
            blk.instructions = out
    return n_new


def build_program(repeats=1, static_exchange=None):
    """Build the SPMD Bass module (same program for all 8 cores).

    static_exchange: replace the partition-id-offset AllGather reads with
    static offsets (pid=0).  Identical instruction/DMA structure (so timing
    is representative) but loss values are only exact on core 0 — used for
    repeat-unrolled timing builds, where the dynamic-offset DMA's
    bounds-check register pairs are exhausted after the first repeat.
    """
    if static_exchange is None:
        static_exchange = repeats > 1
    from contextlib import ExitStack

    import concourse.bass as bass
    import concourse.tile as tile
    from concourse import mybir

    f32 = mybir.dt.float32
    bf16 = mybir.dt.bfloat16
    AF = mybir.ActivationFunctionType
    ALU = mybir.AluOpType

    nc = bass.Bass(
        "TRN2",
        target_bir_lowering=False,
        debug=False,
        num_devices=N_CORES,
        enable_partition_id=True,
    )
    ft_d = nc.dram_tensor("ft", [D, N_ROWS], bf16, kind="ExternalInput")
    mt_d = nc.dram_tensor("mt", [D, N_ROWS], bf16, kind="ExternalInput")
    mg_d = nc.dram_tensor("mg", [D, N_ROWS], bf16, kind="ExternalInput")
    eye_d = nc.dram_tensor("eye", [SUB, SUB], f32, kind="ExternalInput")
    out_d = nc.dram_tensor("partials", [1, 2], f32, kind="ExternalOutput")

    scale = 1.0 / TEMPERATURE
    bias = -1.0 / TEMPERATURE

    with tile.TileContext(nc) as tc:
        with ExitStack() as ctx:
            data = ctx.enter_context(tc.tile_pool(name="data", bufs=1))
            psum = ctx.enter_context(
                tc.tile_pool(name="psum", bufs=2, space="PSUM")
            )
            colp = ctx.enter_context(
                tc.tile_pool(name="colp", bufs=2, space="PSUM")
            )
            scratch = ctx.enter_context(tc.tile_pool(name="scratch", bufs=3))
            statp = ctx.enter_context(tc.tile_pool(name="statp", bufs=1))
            dramp = ctx.enter_context(
                tc.tile_pool(name="dramp", bufs=2, space="DRAM")
            )

            pid = nc.gpsimd.partition_id()
            # AllGather read offsets for the exchanged colsum segments:
            # writer j = (c - d) mod 8, segment d-1.  Snapped once so
            # repeated bodies reuse the same registers.
            ag_offs = {
                d: (
                    ((N_CORES - d) % N_CORES) * XCOLS
                    + (d - 1) * ROWS_PER_CORE
                    if static_exchange
                    else nc.gpsimd.snap(
                        ((pid + (N_CORES - d)) % N_CORES) * XCOLS
                        + (d - 1) * ROWS_PER_CORE
                    )
                )
                for d in (1, 2, 3)
            }

            def body(rep):
                ft_s = data.tile([D, N_ROWS], bf16, name="ft_s")
                mt_s = data.tile([D, N_ROWS], bf16, name="mt_s")
                mg_s = data.tile([D, N_ROWS], bf16, name="mg_s")
                eye_s = data.tile([SUB, SUB], f32, name="eye_s")
                bias_s = data.tile([D, 1], f32, name="bias_s")
                onesb = data.tile([D, 1], bf16, name="onesb")
                onesf = data.tile([D, 1], f32, name="onesf")
                warm = data.tile([D, 1], f32, name="warm")
                nc.vector.memset(bias_s, bias)
                nc.vector.memset(onesb, 1.0)
                nc.vector.memset(onesf, 1.0)
                # warm the ACT exp table while the input DMAs run
                nc.scalar.activation(
                    out=warm, in_=bias_s, func=AF.Exp, bias=0.0, scale=1.0
                )
                # first-unit operands first (small) so compute starts early
                DSL = 2048
                nc.sync.dma_start(out=ft_s[:, 0:UNIT], in_=ft_d[:, 0:UNIT])
                nc.sync.dma_start(out=mg_s[:, 0:UNIT], in_=mg_d[:, 0:UNIT])
                nc.sync.dma_start(
                    out=ft_s[:, UNIT:DSL], in_=ft_d[:, UNIT:DSL]
                )
                nc.sync.dma_start(
                    out=mg_s[:, UNIT:DSL], in_=mg_d[:, UNIT:DSL]
                )
                for k in range(1, N_ROWS // DSL):
                    sl = slice(k * DSL, (k + 1) * DSL)
                    nc.sync.dma_start(out=mg_s[:, sl], in_=mg_d[:, sl])
                    nc.sync.dma_start(out=ft_s[:, sl], in_=ft_d[:, sl])
                for k in range(N_ROWS // DSL):
                    sl = slice(k * DSL, (k + 1) * DSL)
                    nc.sync.dma_start(out=mt_s[:, sl], in_=mt_d[:, sl])
                nc.sync.dma_start(out=eye_s, in_=eye_d[:, :])

                # stats[p, strip, b, unit] = rowsum of exp over that unit
                stats = statp.tile([D, 3, NB, NAU], f32, name="stats")
                csvec = statp.tile([1, N_ROWS], f32, name="csvec")

                def sim_unit(istrip, anchor, target, b, unit, col0, masked):
                    """matmul 128 anchors x UNIT targets -> exp -> rowsums;
                    returns the bf16 exp tile."""
                    lhsT = anchor[:, b * SUB : (b + 1) * SUB]
                    ps = psum.tile([D, UNIT], f32, name="ps", tag="ps")
                    for s in range(UNIT // MM_N):
                        nc.tensor.matmul(
                            ps[:, s * MM_N : (s + 1) * MM_N],
                            lhsT,
                            target[:, col0 + s * MM_N : col0 + (s + 1) * MM_N],
                            start=True,
                            stop=True,
                        )
                    if masked and col0 == 0:
                        w = b * SUB
                        nc.vector.scalar_tensor_tensor(
                            out=ps[:, w : w + SUB],
                            in0=ps[:, w : w + SUB],
                            scalar=1.0,
                            in1=eye_s,
                            op0=ALU.bypass,
                            op1=ALU.subtract,
                        )
                    sc = scratch.tile([D, UNIT], bf16, name="sc")
                    if DVE_ROWSUM:
                        nc.scalar.activation(
                            out=sc, in_=ps, func=AF.Exp,
                            bias=bias_s, scale=scale,
                        )
                        nc.vector.tensor_reduce(
                            out=stats[:, istrip, b, unit : unit + 1],
                            in_=sc,
                            axis=mybir.AxisListType.X,
                            op=ALU.add,
                        )
                    else:
                        nc.scalar.activation(
                            out=sc,
                            in_=ps,
                            func=AF.Exp,
                            bias=bias_s,
                            scale=scale,
                            accum_out=stats[:, istrip, b, unit : unit + 1],
                        )
                    return sc

                def col_mms(colacc, sc, b):
                    """ones-matmul column-sum of one exp tile, accumulated
                    over sub-blocks into a dedicated psum region."""
                    for s in range(UNIT // MM_N):
                        nc.tensor.matmul(
                            colacc[0:1, s * MM_N : (s + 1) * MM_N],
                            onesb,
                            sc[:, s * MM_N : (s + 1) * MM_N],
                            start=(b == 0),
                            stop=(b == NB - 1),
                        )

                def unit_loop(istrip, anchor, target, unit, col0, masked,
                              colsum_vec=None, vec_off=0, after_b0=None):
                    """One 1024-col unit for all 8 sub-blocks.  Colsum
                    matmuls lag one sub-block so the in-order PE never makes
                    the ACT wait on exp output."""
                    colacc = None
                    if colsum_vec is not None:
                        colacc = colp.tile(
                            [1, UNIT], f32, name="colacc", tag="colacc"
                        )
                    prev_sc = None
                    for b in range(NB):
                        sc = sim_unit(istrip, anchor, target, b, unit, col0,
                                      masked)
                        if colacc is not None and prev_sc is not None:
                            col_mms(colacc, prev_sc, b - 1)
                        prev_sc = sc
                        if b == 0 and after_b0 is not None:
                            after_b0()
                    if colacc is not None:
                        col_mms(colacc, prev_sc, NB - 1)
                        nc.vector.tensor_copy(
                            colsum_vec[:, vec_off : vec_off + UNIT],
                            colacc[0:1, :],
                        )

                # --- strip A: F anchors vs global-order M columns -------
                for k in range(NAU):
                    unit_loop(0, ft_s, mg_s, k, k * UNIT, masked=False,
                              colsum_vec=csvec, vec_off=k * UNIT)

                def a_rs():
                    cc_in = dramp.tile([1, N_ROWS], f32, name="cc_in")
                    cc_out = dramp.tile([1, ROWS_PER_CORE], f32, name="cc_out")
                    nc.gpsimd.dma_start(out=cc_in, in_=csvec)
                    nc.gpsimd.collective_compute(
                        "ReduceScatter",
                        mybir.AluOpType.add,
                        replica_groups=[list(range(N_CORES))],
                        ins=[cc_in[:, :]],
                        outs=[cc_out[:, :]],
                    )
                    # core's Q, laid out [part p, sub-block b] = rs[128b+p]
                    qsb = statp.tile([D, NB], f32, name="qsb")
                    nc.gpsimd.dma_start(
                        out=qsb,
                        in_=cc_out.rearrange("o (b p) -> p (o b)", p=SUB),
                    )
                    return qsb

                # --- strips B/C: symmetric, 5 units (d=0..4) each -------
                # Exchange units (d=1..3) run first so each strip's
                # AllGather flies under later exp work; the d=0 (masked
                # diagonal) and d=4 units close out each strip.
                def bc_exchange(istrip, vec):
                    cc_in = dramp.tile([1, XCOLS], f32, name="bc_in")
                    ag = dramp.tile(
                        [1, N_CORES * XCOLS], f32, name="bc_ag",
                        addr_space="Shared",
                    )
                    nc.gpsimd.dma_start(out=cc_in, in_=vec)
                    nc.gpsimd.collective_compute(
                        "AllGather",
                        mybir.AluOpType.bypass,
                        replica_groups=[list(range(N_CORES))],
                        ins=[cc_in[:, :]],
                        outs=[ag[:, :]],
                    )
                    rcvs = []
                    for d in (1, 2, 3):
                        rcv = statp.tile(
                            [D, NB], f32, name="rcv", tag=f"rcv{istrip}{d}"
                        )
                        nc.gpsimd.dma_start(
                            out=rcv,
                            in_=ag[
                                0:1, bass.ds(ag_offs[d], ROWS_PER_CORE)
                            ].rearrange("o (b p) -> p (o b)", p=SUB),
                        )
                        rcvs.append(rcv)
                    return rcvs

                def bc_x_units(istrip, anchor, vec, after_b0=None):
                    # units d=1..3 (cols [1024, 4096)): rowsums + colsums
                    for d in (1, 2, 3):
                        unit_loop(
                            istrip, anchor, anchor, d, d * UNIT, masked=False,
                            colsum_vec=vec, vec_off=(d - 1) * UNIT,
                            after_b0=after_b0 if d == 1 else None,
                        )

                def bc_rest(istrip, anchor):
                    # d=0 (masked diagonal) and d=4 (both-endpoint) units
                    unit_loop(istrip, anchor, anchor, 0, 0, masked=True)
                    unit_loop(istrip, anchor, anchor, 4, 4 * UNIT,
                              masked=False)

                qsb_box = []
                vecC = statp.tile([1, XCOLS], f32, name="vecC")
                vecB = statp.tile([1, XCOLS], f32, name="vecB")
                bc_x_units(2, mt_s, vecC,
                           after_b0=lambda: qsb_box.append(a_rs()))
                rcvC = bc_exchange(2, vecC)
                bc_x_units(1, ft_s, vecB)
                rcvB = bc_exchange(1, vecB)
                bc_rest(2, mt_s)
                bc_rest(1, ft_s)
                qsb = qsb_box[0]

                # --- finale: per-row sums -> per-core partial scalars ----
                sums = statp.tile([D, 3, NB], f32, name="sums")
                nc.vector.tensor_reduce(
                    out=sums[:, 0, :], in_=stats[:, 0, :, :],
                    axis=mybir.AxisListType.X, op=ALU.add,
                )
                for istrip in (1, 2):
                    nc.vector.tensor_reduce(
                        out=sums[:, istrip, :],
                        in_=stats[:, istrip, :, 0:NBCU],
                        axis=mybir.AxisListType.X, op=ALU.add,
                    )
                # fold in the exchanged colsum segments
                for istrip, rcvs in ((1, rcvB), (2, rcvC)):
                    for rcv in rcvs:
                        nc.vector.scalar_tensor_tensor(
                            out=sums[:, istrip, :], in0=sums[:, istrip, :],
                            scalar=1.0, in1=rcv,
                            op0=ALU.bypass, op1=ALU.add,
                        )
                den = statp.tile([D, 2, NB], f32, name="den")
                nc.vector.scalar_tensor_tensor(
                    out=den[:, 0, :], in0=sums[:, 0, :], scalar=1.0,
                    in1=sums[:, 1, :], op0=ALU.bypass, op1=ALU.add,
                )
                nc.vector.scalar_tensor_tensor(
                    out=den[:, 1, :], in0=qsb, scalar=1.0,
                    in1=sums[:, 2, :], op0=ALU.bypass, op1=ALU.add,
                )
                # term sums per partition: sum_b [ln(den) - ln(num)]
                lnacc = statp.tile([D, 4], f32, name="lnacc")
                lnscr = statp.tile([D, NB], f32, name="lnscr")
                for i, src in enumerate(
                    [den[:, 0, :], sums[:, 0, :], den[:, 1, :], qsb]
                ):
                    nc.scalar.activation(
                        out=lnscr, in_=src, func=AF.Ln,
                        accum_out=lnacc[:, i : i + 1],
                    )
                term = statp.tile([D, 2], f32, name="term")
                nc.vector.scalar_tensor_tensor(
                    out=term[:, 0:1], in0=lnacc[:, 0:1], scalar=1.0,
                    in1=lnacc[:, 1:2], op0=ALU.bypass, op1=ALU.subtract,
                )
                nc.vector.scalar_tensor_tensor(
                    out=term[:, 1:2], in0=lnacc[:, 2:3], scalar=1.0,
                    in1=lnacc[:, 3:4], op0=ALU.bypass, op1=ALU.subtract,
                )
                # reduce across the 128 partitions with a ones-matmul
                pfin = psum.tile([D, UNIT], f32, name="pfin", tag="ps")
                nc.tensor.matmul(
                    pfin[0:1, 0:2], onesf, term, start=True, stop=True
                )
                res = statp.tile([1, 2], f32, name="res")
                nc.vector.tensor_copy(res, pfin[0:1, 0:2])
                nc.gpsimd.dma_start(out=out_d[:, :], in_=res)

            for rep in range(repeats):
                body(rep)

    _split_waits(nc, mybir)
    return nc


def make_in_maps(proj_f, proj_m):
    import ml_dtypes

    ftT = np.ascontiguousarray(proj_f.astype(np.float32).T)  # [D, N]
    mtT = np.ascontiguousarray(proj_m.astype(np.float32).T)
    mg = mtT.astype(ml_dtypes.bfloat16)
    eye = MASK_BIG * np.eye(SUB, dtype=np.float32)
    in_maps = []
    for c in range(N_CORES):
        shift = c * ROWS_PER_CORE
        ftc = np.roll(ftT, -shift, axis=1).astype(ml_dtypes.bfloat16)
        mtc = np.roll(mtT, -shift, axis=1).astype(ml_dtypes.bfloat16)
        in_maps.append({"ft": ftc, "mt": mtc, "mg": mg, "eye": eye})
    return in_maps


def combine_partials(results):
    sum_f = 0.0
    sum_m = 0.0
    for r in results:
        p = np.asarray(r["partials"], dtype=np.float64)
        sum_f += p[0, 0]
        sum_m += p[0, 1]
    loss = ASYM_F * (sum_f / N_ROWS) + ASYM_M * (sum_m / N_ROWS)
    return np.float32(loss)


def kernel(proj_f, proj_m):
    from concourse.bass_utils import run_bass_kernel_spmd

    nc = build_program(repeats=1)
    in_maps = make_in_maps(proj_f, proj_m)
    res = run_bass_kernel_spmd(
        nc, in_maps, core_ids=list(range(N_CORES)), trace=False
    )
    return combine_partials(res.results)


# revision 36
# speedup vs baseline: 1.2700x; 1.2597x over previous
"""CrossGender InfoNCE loss on 8 trn2 NeuronCores.

Math: for direction f->m (anchor rows f_i, positives m_j, all L2-normalized):
    P_i = sum_j exp((f_i . m_j - 1)/T)        (numerator sum, shifted)
    N_i = sum_{j!=i} exp((f_i . f_j - 1)/T)   (masked self-sim sum, shifted)
    per-row term = log(P_i + N_i) - log(P_i)  (shift cancels in the ratio)
    loss_f2m = mean_i term_i ; loss = 1.5*loss_f2m + 0.5*loss_m2f

Sharding: core c owns anchor rows [1024c, 1024c+1024) of both proj_f and
proj_m.  Inputs are shipped transposed ([D=128, N=8192], bf16).  ft/mt are
column-rotated by 1024c so each core's own block sits at column 0; the
self-similarity diagonal then lands at a uniform (core-independent)
position, so one SPMD program masks it exactly (subtract 30000*I on PSUM
before exp).  The ACT (exp) engine is the bottleneck (1 elem/lane/cycle +
~350cyc/instruction), so exp work is shared three ways:

* strip A (F.M^T, unrotated M columns) is computed once: row-sums give P;
  column-sums are accumulated on the PE (ones-matmuls over the bf16 exp
  tiles into dedicated PSUM banks) and ReduceScattered (add) across cores —
  in global column order rank c's RS shard is exactly Q for its own
  M-anchor rows.
* strips B (F.F^T) and C (M.M^T) are symmetric: each core computes only
  rotated column blocks d=0..4 (5/8 of the row); blocks d=1..3 also feed
  column-sums that are AllGathered, and each core fetches the 3 segments
  addressed to it (partition-id register offsets) to complete its row-sums.
  Distance-4 blocks are computed by both endpoint cores (rowsum only), so
  nothing is exchanged for them.

All compute runs in 1024-column units: sim psum ping-pong = 2x2 banks,
colsum accumulators = 2x2 banks (8 total).  Colsum matmuls are emitted one
sub-block behind the sim matmuls so the in-order PE never makes the ACT
wait.  Exchange collectives fly under later strips' exp work.  Each core
emits two partial sums; the host combines 8x2 scalars.
"""

import numpy as np

N_ROWS = 8192
D = 128
N_CORES = 8
ROWS_PER_CORE = N_ROWS // N_CORES  # 1024
SUB = 128                          # anchor sub-block (psum partition dim)
NB = ROWS_PER_CORE // SUB          # 8 sub-blocks per core
UNIT = 1024                        # columns per ACT pass (2 psum banks)
MM_N = 512                         # matmul moving free dim
NAU = N_ROWS // UNIT               # 8 units (strip A)
NBCU = 5                           # strip B/C units (d=0..4)
XCOLS = 3 * ROWS_PER_CORE          # exchanged colsum width (d=1..3)
TEMPERATURE = 0.07
ASYM_F = 1.5
ASYM_M = 0.5
MASK_BIG = 30000.0
DVE_ROWSUM = False  # rowsums on the DVE instead of ACT accum_out (A/B knob)
SCRATCH_BUFS = 6                   # exp-output tile slots
COL_LAG = 1                        # colsum matmul lag behind sim matmuls


def _split_waits(nc, mybir, maxw=1):
    """Workaround: this walrus build rejects >1 sync-wait on one instruction
    ("Too many sync wait commands").  Hoist extras onto preceding NoOps on
    the same engine (engines execute sequentially, so semantics keep)."""
    fn = nc.m.functions[0]
    n_new = 0
    for blk in fn.blocks:
        out = []
        changed = False
        for inst in blk.instructions:
            si = inst.sync_info
            if si is not None and si.on_wait and len(si.on_wait) > maxw:
                waits = list(si.on_wait)
                for w in waits[:-maxw]:
                    nop = mybir.InstNoOp(
                        name=f"{inst.name}-w{n_new}", ins=[], outs=[]
                    )
                    n_new += 1
                    nop.engine = inst.engine
                    nop.sync_info = mybir.SyncInfo(on_wait=[w], on_update=[])
                    out.append(nop)
                inst.sync_info = mybir.SyncInfo(
                    on_wait=waits[-maxw:], on_update=list(si.on_update)
                )
                changed = True
            out.append(inst)
        if changed:
            blk.instructions = out
    return n_new


def build_program(repeats=1, static_exchange=None):
    """Build the SPMD Bass module (same program for all 8 cores).

    static_exchange: replace the partition-id-offset AllGather reads with
    static offsets (pid=0).  Identical instruction/DMA structure (so timing
    is representative) but loss values are only exact on core 0 — used for
    repeat-unrolled timing builds, where the dynamic-offset DMA's
    bounds-check register pairs are exhausted after the first repeat.
    """
    if static_exchange is None:
        static_exchange = repeats > 1
    from contextlib import ExitStack

    import concourse.bass as bass
    import concourse.tile as tile
    from concourse import mybir

    f32 = mybir.dt.float32
    bf16 = mybir.dt.bfloat16
    AF = mybir.ActivationFunctionType
    ALU = mybir.AluOpType

    nc = bass.Bass(
        "TRN2",
        target_bir_lowering=False,
        debug=False,
        num_devices=N_CORES,
        enable_partition_id=True,
    )
    ft_d = nc.dram_tensor("ft", [D, N_ROWS], bf16, kind="ExternalInput")
    mt_d = nc.dram_tensor("mt", [D, N_ROWS], bf16, kind="ExternalInput")
    mg_d = nc.dram_tensor("mg", [D, N_ROWS], bf16, kind="ExternalInput")
    eye_d = nc.dram_tensor("eye", [SUB, SUB], f32, kind="ExternalInput")
    out_d = nc.dram_tensor("partials", [1, 2], f32, kind="ExternalOutput")

    scale = 1.0 / TEMPERATURE
    bias = -1.0 / TEMPERATURE

    with tile.TileContext(nc) as tc:
        with ExitStack() as ctx:
            data = ctx.enter_context(tc.tile_pool(name="data", bufs=1))
            psum = ctx.enter_context(
                tc.tile_pool(name="psum", bufs=3, space="PSUM")
            )
            colp = ctx.enter_context(
                tc.tile_pool(name="colp", bufs=1, space="PSUM")
            )
            scratch = ctx.enter_context(tc.tile_pool(name="scratch", bufs=SCRATCH_BUFS))
            statp = ctx.enter_context(tc.tile_pool(name="statp", bufs=1))
            dramp = ctx.enter_context(
                tc.tile_pool(name="dramp", bufs=2, space="DRAM")
            )

            pid = nc.gpsimd.partition_id()
            # AllGather read offsets for the exchanged colsum segments:
            # writer j = (c - d) mod 8, segment d-1.  Snapped once so
            # repeated bodies reuse the same registers.
            ag_offs = {
                d: (
                    ((N_CORES - d) % N_CORES) * XCOLS
                    + (d - 1) * ROWS_PER_CORE
                    if static_exchange
                    else nc.gpsimd.snap(
                        ((pid + (N_CORES - d)) % N_CORES) * XCOLS
                        + (d - 1) * ROWS_PER_CORE
                    )
                )
                for d in (1, 2, 3)
            }

            def body(rep):
                ft_s = data.tile([D, N_ROWS], bf16, name="ft_s")
                mt_s = data.tile([D, N_ROWS], bf16, name="mt_s")
                mg_s = data.tile([D, N_ROWS], bf16, name="mg_s")
                eye_s = data.tile([SUB, SUB], f32, name="eye_s")
                bias_s = data.tile([D, 1], f32, name="bias_s")
                onesb = data.tile([D, 1], bf16, name="onesb")
                onesf = data.tile([D, 1], f32, name="onesf")
                warm = data.tile([D, 1], f32, name="warm")
                nc.vector.memset(bias_s, bias)
                nc.vector.memset(onesb, 1.0)
                nc.vector.memset(onesf, 1.0)
                # warm the ACT exp table while the input DMAs run
                nc.scalar.activation(
                    out=warm, in_=bias_s, func=AF.Exp, bias=0.0, scale=1.0
                )
                # first-unit operands first (small) so compute starts early
                DSL = 2048
                nc.sync.dma_start(out=ft_s[:, 0:UNIT], in_=ft_d[:, 0:UNIT])
                nc.sync.dma_start(out=mg_s[:, 0:UNIT], in_=mg_d[:, 0:UNIT])
                nc.sync.dma_start(
                    out=ft_s[:, UNIT:DSL], in_=ft_d[:, UNIT:DSL]
                )
                nc.sync.dma_start(
                    out=mg_s[:, UNIT:DSL], in_=mg_d[:, UNIT:DSL]
                )
                for k in range(1, N_ROWS // DSL):
                    sl = slice(k * DSL, (k + 1) * DSL)
                    nc.sync.dma_start(out=mg_s[:, sl], in_=mg_d[:, sl])
                    nc.sync.dma_start(out=ft_s[:, sl], in_=ft_d[:, sl])
                for k in range(N_ROWS // DSL):
                    sl = slice(k * DSL, (k + 1) * DSL)
                    nc.sync.dma_start(out=mt_s[:, sl], in_=mt_d[:, sl])
                nc.sync.dma_start(out=eye_s, in_=eye_d[:, :])

                # stats[p, strip, b, unit] = rowsum of exp over that unit
                stats = statp.tile([D, 3, NB, NAU], f32, name="stats")
                csvec = statp.tile([1, N_ROWS], f32, name="csvec")

                def sim_unit(istrip, anchor, target, b, unit, col0, masked):
                    """matmul 128 anchors x UNIT targets -> exp -> rowsums;
                    returns the bf16 exp tile."""
                    lhsT = anchor[:, b * SUB : (b + 1) * SUB]
                    ps = psum.tile([D, UNIT], f32, name="ps", tag="ps")
                    for s in range(UNIT // MM_N):
                        nc.tensor.matmul(
                            ps[:, s * MM_N : (s + 1) * MM_N],
                            lhsT,
                            target[:, col0 + s * MM_N : col0 + (s + 1) * MM_N],
                            start=True,
                            stop=True,
                        )
                    if masked and col0 == 0:
                        w = b * SUB
                        nc.vector.scalar_tensor_tensor(
                            out=ps[:, w : w + SUB],
                            in0=ps[:, w : w + SUB],
                            scalar=1.0,
                            in1=eye_s,
                            op0=ALU.bypass,
                            op1=ALU.subtract,
                        )
                    sc = scratch.tile([D, UNIT], bf16, name="sc")
                    if DVE_ROWSUM:
                        nc.scalar.activation(
                            out=sc, in_=ps, func=AF.Exp,
                            bias=bias_s, scale=scale,
                        )
                        nc.vector.tensor_reduce(
                            out=stats[:, istrip, b, unit : unit + 1],
                            in_=sc,
                            axis=mybir.AxisListType.X,
                            op=ALU.add,
                        )
                    else:
                        nc.scalar.activation(
                            out=sc,
                            in_=ps,
                            func=AF.Exp,
                            bias=bias_s,
                            scale=scale,
                            accum_out=stats[:, istrip, b, unit : unit + 1],
                        )
                    return sc

                def col_mms(colacc, sc, b):
                    """ones-matmul column-sum of one exp tile, accumulated
                    over sub-blocks into a dedicated psum region."""
                    for s in range(UNIT // MM_N):
                        nc.tensor.matmul(
                            colacc[0:1, s * MM_N : (s + 1) * MM_N],
                            onesb,
                            sc[:, s * MM_N : (s + 1) * MM_N],
                            start=(b == 0),
                            stop=(b == NB - 1),
                        )

                def unit_loop(istrip, anchor, target, unit, col0, masked,
                              colsum_vec=None, vec_off=0, after_b0=None):
                    """One 1024-col unit for all 8 sub-blocks.  Colsum
                    matmuls lag one sub-block so the in-order PE never makes
                    the ACT wait on exp output."""
                    colacc = None
                    if colsum_vec is not None:
                        colacc = colp.tile(
                            [1, UNIT], f32, name="colacc", tag="colacc"
                        )
                    pend = []
                    for b in range(NB):
                        sc = sim_unit(istrip, anchor, target, b, unit, col0,
                                      masked)
                        if colacc is not None:
                            pend.append((sc, b))
                            if len(pend) > COL_LAG:
                                psc, pb = pend.pop(0)
                                col_mms(colacc, psc, pb)
                        if b == 0 and after_b0 is not None:
                            after_b0()
                    if colacc is not None:
                        for psc, pb in pend:
                            col_mms(colacc, psc, pb)
                        nc.vector.tensor_copy(
                            colsum_vec[:, vec_off : vec_off + UNIT],
                            colacc[0:1, :],
                        )

                # --- strip A: F anchors vs global-order M columns -------
                for k in range(NAU):
                    unit_loop(0, ft_s, mg_s, k, k * UNIT, masked=False,
                              colsum_vec=csvec, vec_off=k * UNIT)

                def a_rs():
                    cc_in = dramp.tile([1, N_ROWS], f32, name="cc_in")
                    cc_out = dramp.tile([1, ROWS_PER_CORE], f32, name="cc_out")
                    nc.gpsimd.dma_start(out=cc_in, in_=csvec)
                    nc.gpsimd.collective_compute(
                        "ReduceScatter",
                        mybir.AluOpType.add,
                        replica_groups=[list(range(N_CORES))],
                        ins=[cc_in[:, :]],
                        outs=[cc_out[:, :]],
                    )
                    # core's Q, laid out [part p, sub-block b] = rs[128b+p]
                    qsb = statp.tile([D, NB], f32, name="qsb")
                    nc.gpsimd.dma_start(
                        out=qsb,
                        in_=cc_out.rearrange("o (b p) -> p (o b)", p=SUB),
                    )
                    return qsb

                # --- strips B/C: symmetric, 5 units (d=0..4) each -------
                # Exchange units (d=1..3) run first so each strip's
                # AllGather flies under later exp work; the d=0 (masked
                # diagonal) and d=4 units close out each strip.
                def bc_exchange(istrip, vec):
                    cc_in = dramp.tile([1, XCOLS], f32, name="bc_in")
                    ag = dramp.tile(
                        [1, N_CORES * XCOLS], f32, name="bc_ag",
                        addr_space="Shared",
                    )
                    nc.gpsimd.dma_start(out=cc_in, in_=vec)
                    nc.gpsimd.collective_compute(
                        "AllGather",
                        mybir.AluOpType.bypass,
                        replica_groups=[list(range(N_CORES))],
                        ins=[cc_in[:, :]],
                        outs=[ag[:, :]],
                    )
                    rcvs = []
                    for d in (1, 2, 3):
                        rcv = statp.tile(
                            [D, NB], f32, name="rcv", tag=f"rcv{istrip}{d}"
                        )
                        nc.gpsimd.dma_start(
                            out=rcv,
                            in_=ag[
                                0:1, bass.ds(ag_offs[d], ROWS_PER_CORE)
                            ].rearrange("o (b p) -> p (o b)", p=SUB),
                        )
                        rcvs.append(rcv)
                    return rcvs

                def bc_x_units(istrip, anchor, vec, after_b0=None):
                    # units d=1..3 (cols [1024, 4096)): rowsums + colsums
                    for d in (1, 2, 3):
                        unit_loop(
                            istrip, anchor, anchor, d, d * UNIT, masked=False,
                            colsum_vec=vec, vec_off=(d - 1) * UNIT,
                            after_b0=after_b0 if d == 1 else None,
                        )

                def bc_rest(istrip, anchor):
                    # d=0 (masked diagonal) and d=4 (both-endpoint) units
                    unit_loop(istrip, anchor, anchor, 0, 0, masked=True)
                    unit_loop(istrip, anchor, anchor, 4, 4 * UNIT,
                              masked=False)

                qsb_box = []
                vecC = statp.tile([1, XCOLS], f32, name="vecC")
                vecB = statp.tile([1, XCOLS], f32, name="vecB")
                bc_x_units(2, mt_s, vecC,
                           after_b0=lambda: qsb_box.append(a_rs()))
                rcvC = bc_exchange(2, vecC)
                bc_x_units(1, ft_s, vecB)
                rcvB = bc_exchange(1, vecB)
                bc_rest(2, mt_s)
                bc_rest(1, ft_s)
                qsb = qsb_box[0]

                # --- finale: per-row sums -> per-core partial scalars ----
                sums = statp.tile([D, 3, NB], f32, name="sums")
                nc.vector.tensor_reduce(
                    out=sums[:, 0, :], in_=stats[:, 0, :, :],
                    axis=mybir.AxisListType.X, op=ALU.add,
                )
                for istrip in (1, 2):
                    nc.vector.tensor_reduce(
                        out=sums[:, istrip, :],
                        in_=stats[:, istrip, :, 0:NBCU],
                        axis=mybir.AxisListType.X, op=ALU.add,
                    )
                # fold in the exchanged colsum segments
                for istrip, rcvs in ((1, rcvB), (2, rcvC)):
                    for rcv in rcvs:
                        nc.vector.scalar_tensor_tensor(
                            out=sums[:, istrip, :], in0=sums[:, istrip, :],
                            scalar=1.0, in1=rcv,
                            op0=ALU.bypass, op1=ALU.add,
                        )
                den = statp.tile([D, 2, NB], f32, name="den")
                nc.vector.scalar_tensor_tensor(
                    out=den[:, 0, :], in0=sums[:, 0, :], scalar=1.0,
                    in1=sums[:, 1, :], op0=ALU.bypass, op1=ALU.add,
                )
                nc.vector.scalar_tensor_tensor(
                    out=den[:, 1, :], in0=qsb, scalar=1.0,
                    in1=sums[:, 2, :], op0=ALU.bypass, op1=ALU.add,
                )
                # term sums per partition: sum_b [ln(den) - ln(num)]
                lnacc = statp.tile([D, 4], f32, name="lnacc")
                lnscr = statp.tile([D, NB], f32, name="lnscr")
                for i, src in enumerate(
                    [den[:, 0, :], sums[:, 0, :], den[:, 1, :], qsb]
                ):
                    nc.scalar.activation(
                        out=lnscr, in_=src, func=AF.Ln,
                        accum_out=lnacc[:, i : i + 1],
                    )
                term = statp.tile([D, 2], f32, name="term")
                nc.vector.scalar_tensor_tensor(
                    out=term[:, 0:1], in0=lnacc[:, 0:1], scalar=1.0,
                    in1=lnacc[:, 1:2], op0=ALU.bypass, op1=ALU.subtract,
                )
                nc.vector.scalar_tensor_tensor(
                    out=term[:, 1:2], in0=lnacc[:, 2:3], scalar=1.0,
                    in1=lnacc[:, 3:4], op0=ALU.bypass, op1=ALU.subtract,
                )
                # reduce across the 128 partitions with a ones-matmul
                pfin = psum.tile([D, UNIT], f32, name="pfin", tag="ps")
                nc.tensor.matmul(
                    pfin[0:1, 0:2], onesf, term, start=True, stop=True
                )
                res = statp.tile([1, 2], f32, name="res")
                nc.vector.tensor_copy(res, pfin[0:1, 0:2])
                nc.gpsimd.dma_start(out=out_d[:, :], in_=res)

            for rep in range(repeats):
                body(rep)

    _split_waits(nc, mybir)
    return nc


def make_in_maps(proj_f, proj_m):
    import ml_dtypes

    ftT = np.ascontiguousarray(proj_f.astype(np.float32).T)  # [D, N]
    mtT = np.ascontiguousarray(proj_m.astype(np.float32).T)
    mg = mtT.astype(ml_dtypes.bfloat16)
    eye = MASK_BIG * np.eye(SUB, dtype=np.float32)
    in_maps = []
    for c in range(N_CORES):
        shift = c * ROWS_PER_CORE
        ftc = np.roll(ftT, -shift, axis=1).astype(ml_dtypes.bfloat16)
        mtc = np.roll(mtT, -shift, axis=1).astype(ml_dtypes.bfloat16)
        in_maps.append({"ft": ftc, "mt": mtc, "mg": mg, "eye": eye})
    return in_maps


def combine_partials(results):
    sum_f = 0.0
    sum_m = 0.0
    for r in results:
        p = np.asarray(r["partials"], dtype=np.float64)
        sum_f += p[0, 0]
        sum_m += p[0, 1]
    loss = ASYM_F * (sum_f / N_ROWS) + ASYM_M * (sum_m / N_ROWS)
    return np.float32(loss)


def kernel(proj_f, proj_m):
    from concourse.bass_utils import run_bass_kernel_spmd

    nc = build_program(repeats=1)
    in_maps = make_in_maps(proj_f, proj_m)
    res = run_bass_kernel_spmd(
        nc, in_maps, core_ids=list(range(N_CORES)), trace=False
    )
    return combine_partials(res.results)
